# revision 27
# baseline (speedup 1.0000x reference)
"""Bass/Trainium2 kernel for nn_Attention_28140625723842.

Multi-head attention (B=2, S=2048, D=1024, H=16, DH=64) with key-padding
mask, sharded over 8 NeuronCores as 2 batches x 4 head-groups (tensor
parallel over heads, data parallel over batch).

Per-core strategy:
  - Host passes transposed activations qT/kT/vT [D, *] (bf16) so the
    d-contraction sits on SBUF partitions; k/v token columns are gathered
    down to the unmasked set (padded to a multiple of 128) — masked keys
    contribute exactly 0 to softmax numerator and denominator, so the
    result is unchanged while scores/exp/PV work halves.
  - Projections produce QT/KT transposed [dh, tokens] (2 heads stacked per
    128 partitions) and V natural [tokens, 4*(dh+1)] with a ones column
    per head.
  - scoresT[k, q] = KT_tile.T @ QT; the pad-key mask becomes a
    per-partition bias fused into the ScalarE exp:
    expS = exp(scores/sqrt(D) + (-1e9)*(1-mask)).
  - P@V uses lhsT = [V_h | 1] so the softmax denominator (row sum) comes
    out as column dh of the matmul output; a PE transpose brings each
    [65, 128] block to [q, 65] layout, where DVE reciprocal +
    tensor_scalar_mul normalize straight into the fp32 output buffer.
  - All matmul operands are bf16 (PSUM accumulation fp32); output fp32.
"""

import numpy as np

B, S, D, H = 2, 2048, 1024, 16
DH = D // H            # 64 head dim
NCORES = 8
GROUPS = NCORES // B   # 4 head groups
HL = H // GROUPS       # 4 heads per core
GW = HL * DH           # 256 output columns per core

P = 128
ND = D // P            # 8 contraction tiles
NT = S // P            # 16 q token tiles
QB = 1024              # q block (one exp op width)
NQB = S // QB          # 2
CH = 512               # matmul free-dim chunk (one PSUM bank fp32)
NCH = QB // CH         # 2

COMPACT = True         # gather unmasked k/v tokens on host

_CACHE = {}


def _chunks(total, width):
    out = []
    o = 0
    while o < total:
        w = min(width, total - o)
        out.append((o, w))
        o += w
    return out


def _build_nc(nk, use_bias=True, debug=False):
    import concourse.bacc as bacc
    import concourse.mybir as mybir
    import concourse.tile as tile
    from concourse.masks import make_identity

    f32 = mybir.dt.float32
    bf16 = mybir.dt.bfloat16
    i32 = mybir.dt.int32
    Exp = mybir.ActivationFunctionType.Exp
    SCALE = float(1.0 / np.sqrt(np.float32(D)))
    NTK = nk // P          # k token tiles (compacted)

    nc = bacc.Bacc(None, target_bir_lowering=False)
    qt_d = nc.dram_tensor("qt", [D, S], bf16, kind="ExternalInput")
    kt_d = nc.dram_tensor("kt", [D, nk], bf16, kind="ExternalInput")
    vt_d = nc.dram_tensor("vt", [D, nk], bf16, kind="ExternalInput")
    wq_d = nc.dram_tensor("wq", [D, GW], bf16, kind="ExternalInput")
    wk_d = nc.dram_tensor("wk", [D, GW], bf16, kind="ExternalInput")
    wv_d = nc.dram_tensor("wv", [D, GW], bf16, kind="ExternalInput")
    bq_d = nc.dram_tensor("bq", [GW], bf16, kind="ExternalInput")
    bk_d = nc.dram_tensor("bk", [GW], bf16, kind="ExternalInput")
    bv_d = nc.dram_tensor("bv", [GW], bf16, kind="ExternalInput")
    mask_d = nc.dram_tensor("mask", [nk], i32, kind="ExternalInput")
    out_d = nc.dram_tensor("out", [S, GW], f32, kind="ExternalOutput")
    if debug:
        dbg_qt = nc.dram_tensor("dbg_qt", [P, HL // 2, S], bf16, kind="ExternalOutput")
        dbg_kt = nc.dram_tensor("dbg_kt", [P, HL // 2, nk], bf16, kind="ExternalOutput")
        dbg_v = nc.dram_tensor("dbg_v", [P, NTK, HL * (DH + 1)], bf16, kind="ExternalOutput")

    with tile.TileContext(nc) as tc:
        with (
            tc.tile_pool(name="consts", bufs=1) as consts,
            tc.tile_pool(name="persist", bufs=1) as persist,
            tc.tile_pool(name="wpool", bufs=2) as wpool,
            tc.tile_pool(name="xt", bufs=3) as xtp,
            tc.tile_pool(name="vx", bufs=ND) as vxp,
            tc.tile_pool(name="exps", bufs=2 * NTK + 2) as expp,
            tc.tile_pool(name="tmp", bufs=3) as tmpp,
            tc.tile_pool(name="rec", bufs=4) as recp,
        ):
            ident = consts.tile([P, P], f32, tag="ident")
            make_identity(nc, ident)
            ones = consts.tile([1, CH], bf16, tag="ones")
            nc.vector.memset(ones, 1.0)

            # mask[k] -> per-partition exp bias: (m - 1) * 1e9  (0 or -1e9)
            maski = consts.tile([P, NTK], i32, tag="maski")
            nc.scalar.dma_start(maski, mask_d.rearrange("(t p) -> p t", p=P))
            maskb = consts.tile([P, NTK], f32, tag="maskb")
            nc.vector.tensor_scalar(
                maskb, maski, -1.0, 1e9,
                mybir.AluOpType.add, mybir.AluOpType.mult,
            )

            brow = {}
            if use_bias:
                for nm, drm in (("q", bq_d), ("k", bk_d), ("v", bv_d)):
                    t = consts.tile([1, GW], bf16, tag=f"bias_{nm}")
                    nc.scalar.dma_start(t, drm[None, :])
                    brow[nm] = t

            QT = persist.tile([P, HL // 2, S], bf16, tag="QT")
            KT = persist.tile([P, HL // 2, nk], bf16, tag="KT")
            V = persist.tile([P, NTK, HL * (DH + 1)], bf16, tag="V")
            V4 = V.rearrange("p t (h e) -> p t h e", h=HL)
            out_sb = persist.tile([P, NT, GW], f32, tag="osb")

            for h in range(HL):
                nc.vector.memset(V4[:, :, h, DH], 1.0)

            with tc.tile_pool(name="pps", bufs=8, space="PSUM") as pps:
                # ---- QT / KT projections: out[dh2, tok] accumulated over d ----
                for nm, xdr, wdr, bkey, OUT, width in (
                    ("q", qt_d, wq_d, "q", QT, S),
                    ("k", kt_d, wk_d, "k", KT, nk),
                ):
                    w_sb = wpool.tile([P, ND, GW], bf16, tag="w")
                    wdr_blk = wdr.rearrange("(n p) w -> p n w", p=P)
                    chs = _chunks(width, CH)
                    pst = {}
                    for dt_ in range(ND):
                        nc.sync.dma_start(w_sb[:, dt_, :], wdr_blk[:, dt_, :])
                        x_sb = xtp.tile([P, S], bf16, tag="xt",
                                        name=f"x_{nm}_{dt_}")
                        if dt_ == 0:
                            half = (len(chs) + 1) // 2 * CH
                            half = min(half, width)
                            nc.sync.dma_start(x_sb[:, :half],
                                              xdr[dt_ * P:(dt_ + 1) * P, :half])
                            if half < width:
                                nc.sync.dma_start(
                                    x_sb[:, half:width],
                                    xdr[dt_ * P:(dt_ + 1) * P, half:])
                        else:
                            nc.sync.dma_start(x_sb[:, :width],
                                              xdr[dt_ * P:(dt_ + 1) * P, :])
                        for hp in range(HL // 2):
                            for ci, (co, cw) in enumerate(chs):
                                if dt_ == 0:
                                    pst[(hp, ci)] = pps.tile(
                                        [P, CH], f32, tag="pp",
                                        name=f"pp_{nm}_{hp}_{ci}")
                                nc.tensor.matmul(
                                    pst[(hp, ci)][:, :cw],
                                    lhsT=w_sb[:, dt_, hp * P:(hp + 1) * P],
                                    rhs=x_sb[:, co:co + cw],
                                    start=(dt_ == 0),
                                    stop=(not use_bias and dt_ == ND - 1),
                                )
                    for hp in range(HL // 2):
                        for ci, (co, cw) in enumerate(chs):
                            if use_bias:
                                nc.tensor.matmul(
                                    pst[(hp, ci)][:, :cw],
                                    lhsT=brow[bkey][:, hp * P:(hp + 1) * P],
                                    rhs=ones[:, :cw],
                                    start=False, stop=True,
                                )
                            nc.vector.tensor_copy(
                                out=OUT[:, hp, co:co + cw],
                                in_=pst[(hp, ci)][:, :cw],
                            )

                # ---- V projection: natural [tok, 4*dh] ----
                # tok-tile outer so each PSUM accumulation group owns a
                # whole bank (start=True clears has_written bank-wide).
                wv_sb = wpool.tile([P, ND, GW], bf16, tag="w")
                nc.sync.dma_start(wv_sb, wv_d.rearrange("(n p) w -> p n w", p=P))
                xvt = []
                for dt_ in range(ND):
                    t = vxp.tile([P, nk], bf16, tag="xvt", name=f"xvt_{dt_}")
                    nc.sync.dma_start(t, vt_d[dt_ * P:(dt_ + 1) * P, :])
                    xvt.append(t)
                for tt in range(NTK):
                    vp_ps = pps.tile([P, GW], f32, tag="pp", name=f"ppv_{tt}")
                    for dt_ in range(ND):
                        nc.tensor.matmul(
                            vp_ps,
                            lhsT=xvt[dt_][:, tt * P:(tt + 1) * P],
                            rhs=wv_sb[:, dt_, :],
                            start=(dt_ == 0),
                            stop=(not use_bias and dt_ == ND - 1),
                        )
                    if use_bias:
                        nc.tensor.matmul(
                            vp_ps,
                            lhsT=ones[:, :P],
                            rhs=brow["v"],
                            start=False, stop=True,
                        )
                    nc.vector.tensor_copy(
                        out=V4[:, tt, :, :DH],
                        in_=vp_ps.rearrange("p (h e) -> p h e", h=HL),
                    )

            # ---- attention ----
            with (
                tc.tile_pool(name="pss", bufs=2, space="PSUM") as pss,
                tc.tile_pool(name="pspv", bufs=2, space="PSUM") as pspv,
                tc.tile_pool(name="pstr", bufs=2, space="PSUM") as pstr,
            ):
                for h in range(HL):
                    hp, ho = divmod(h, 2)
                    po = ho * DH  # partition offset within the stacked pair
                    for qb in range(NQB):
                        etiles = []
                        for kt_ in range(NTK):
                            sps = pss.tile([P, QB], f32, tag="s",
                                           name=f"s_{h}_{qb}_{kt_}")
                            for c in range(NCH):
                                nc.tensor.matmul(
                                    sps[:, c * CH:(c + 1) * CH],
                                    lhsT=KT[po:po + DH, hp, kt_ * P:(kt_ + 1) * P],
                                    rhs=QT[po:po + DH, hp,
                                           qb * QB + c * CH:qb * QB + (c + 1) * CH],
                                    start=True, stop=True,
                                )
                            e = expp.tile([P, QB], bf16, tag="e",
                                          name=f"e_{h}_{qb}_{kt_}")
                            nc.scalar.activation(
                                e, sps, Exp,
                                bias=maskb[:, kt_:kt_ + 1], scale=SCALE,
                            )
                            etiles.append(e)
                        pvt = [pspv.tile([DH + 1, CH], f32, tag="pv",
                                         name=f"pv_{h}_{qb}_{c}")
                               for c in range(NCH)]
                        for kt_ in range(NTK):
                            for c in range(NCH):
                                nc.tensor.matmul(
                                    pvt[c],
                                    lhsT=V[:, kt_, h * (DH + 1):(h + 1) * (DH + 1)],
                                    rhs=etiles[kt_][:, c * CH:(c + 1) * CH],
                                    start=(kt_ == 0), stop=(kt_ == NTK - 1),
                                )
                        out_blk = out_d.rearrange("(t p) w -> p t w", p=P)
                        for c in range(NCH):
                            pv_sb = tmpp.tile([DH + 1, CH], f32, tag="pvsb")
                            nc.vector.tensor_copy(out=pv_sb, in_=pvt[c])
                            for q4 in range(CH // P):
                                tps = pstr.tile([P, DH + 1], f32, tag="tr")
                                nc.tensor.transpose(
                                    tps,
                                    pv_sb[:, q4 * P:(q4 + 1) * P],
                                    ident[:DH + 1, :DH + 1],
                                )
                                rec = recp.tile([P, 1], f32, tag="rec")
                                nc.vector.reciprocal(rec, tps[:, DH:DH + 1])
                                tokt = qb * (QB // P) + c * (CH // P) + q4
                                nc.vector.tensor_scalar_mul(
                                    out_sb[:, tokt, h * DH:(h + 1) * DH],
                                    tps[:, :DH],
                                    rec,
                                )
                            if h == HL - 1:
                                t0 = qb * (QB // P) + c * (CH // P)
                                t1 = t0 + CH // P
                                nc.sync.dma_start(
                                    out_blk[:, t0:t1, :], out_sb[:, t0:t1, :]
                                )

                if debug:
                    nc.sync.dma_start(dbg_qt[:], QT)
                    nc.sync.dma_start(dbg_kt[:], KT)
                    nc.sync.dma_start(dbg_v[:], V)
    nc.compile()
    return nc


def _get_nc(nk, use_bias=True, debug=False):
    key = (nk, use_bias, debug)
    if key not in _CACHE:
        _CACHE[key] = _build_nc(nk, use_bias=use_bias, debug=debug)
    return _CACHE[key]


def _run(nc, in_maps, trace=False):
    from concourse.bass_utils import run_bass_kernel_spmd

    return run_bass_kernel_spmd(
        nc, in_maps, core_ids=list(range(NCORES)), trace=trace
    )


def _make_in_maps(q, k, v, mask, Wq, bq, Wk, bk, Wv, bv):
    import ml_dtypes

    bf16 = ml_dtypes.bfloat16
    q = np.asarray(q, np.float32)
    k = np.asarray(k, np.float32)
    v = np.asarray(v, np.float32)
    mask = np.asarray(mask, np.int32)
    Wq = np.asarray(Wq, np.float32).astype(bf16)
    Wk = np.asarray(Wk, np.float32).astype(bf16)
    Wv = np.asarray(Wv, np.float32).astype(bf16)
    bq = np.asarray(bq, np.float32).astype(bf16)
    bk = np.asarray(bk, np.float32).astype(bf16)
    bv = np.asarray(bv, np.float32).astype(bf16)

    use_bias = bool(
        np.any(np.asarray(bq, np.float32))
        or np.any(np.asarray(bk, np.float32))
        or np.any(np.asarray(bv, np.float32))
    )
    if COMPACT:
        idxs = [np.nonzero(mask[b])[0] for b in range(B)]
        neff = max(1, max(len(ix) for ix in idxs))
        nk = -(-neff // P) * P  # round up to multiple of 128
    else:
        idxs = [np.arange(S) for _ in range(B)]
        nk = S

    qT = [np.ascontiguousarray(q[b].T).astype(bf16) for b in range(B)]
    kT, vT, mk = [], [], []
    for b in range(B):
        ix = idxs[b]
        kc = np.zeros((D, nk), bf16)
        vc = np.zeros((D, nk), bf16)
        kc[:, :len(ix)] = k[b].T[:, ix].astype(bf16)
        vc[:, :len(ix)] = v[b].T[:, ix].astype(bf16)
        kT.append(kc)
        vT.append(vc)
        m = np.zeros((nk,), np.int32)
        if COMPACT:
            m[:len(ix)] = 1
        else:
            m[:] = mask[b]
        mk.append(m)

    in_maps = []
    for c in range(NCORES):
        b, g = divmod(c, GROUPS)
        cols = slice(g * GW, (g + 1) * GW)
        in_maps.append({
            "qt": qT[b],
            "kt": kT[b],
            "vt": vT[b],
            "wq": np.ascontiguousarray(Wq[:, cols]),
            "wk": np.ascontiguousarray(Wk[:, cols]),
            "wv": np.ascontiguousarray(Wv[:, cols]),
            "bq": np.ascontiguousarray(bq[cols]),
            "bk": np.ascontiguousarray(bk[cols]),
            "bv": np.ascontiguousarray(bv[cols]),
            "mask": mk[b],
        })
    return nk, use_bias, in_maps


def _assemble(results):
    out = np.empty((B, S, D), np.float32)
    for c in range(NCORES):
        b, g = divmod(c, GROUPS)
        out[b, :, g * GW:(g + 1) * GW] = results[c]["out"]
    return out


def kernel(q, k, v, mask, Wq, bq, Wk, bk, Wv, bv):
    nk, use_bias, in_maps = _make_in_maps(q, k, v, mask, Wq, bq, Wk, bk, Wv, bv)
    res = _run(_get_nc(nk, use_bias), in_maps, trace=False)
    return _assemble(res.results)


def _install_ntff_hook():
    """The image's antenv stub lacks axon_hooks; synthesize it and register
    the ctypes NTFF hook that trn_agent_boot would have installed."""
    import sys
    import types

    import antenv

    if "antenv.axon_hooks" in sys.modules:
        return
    mod = types.ModuleType("antenv.axon_hooks")
    state = {"hook": None}
    mod.set_axon_ntff_profile_hook = lambda h: state.__setitem__("hook", h)
    mod.get_axon_ntff_profile_hook = lambda: state["hook"]
    sys.modules["antenv.axon_hooks"] = mod
    antenv.axon_hooks = mod
    try:
        from trn_agent_boot.trn_boot import _ntff_profile_via_ctypes

        mod.set_axon_ntff_profile_hook(
            _ntff_profile_via_ctypes("/opt/axon/libaxon_pjrt.so")
        )
    except Exception as e:
        print(f"ntff hook registration failed: {e}")


def kernel_traced(q, k, v, mask, Wq, bq, Wk, bk, Wv, bv):
    """Same as kernel() but also returns (output, exec_time_ns)."""
    _install_ntff_hook()
    nk, use_bias, in_maps = _make_in_maps(q, k, v, mask, Wq, bq, Wk, bk, Wv, bv)
    res = _run(_get_nc(nk, use_bias), in_maps, trace=True)
    return _assemble(res.results), res.exec_time_ns


# revision 31
# speedup vs baseline: 1.0486x; 1.0486x over previous
"""Bass/Trainium2 kernel for nn_Attention_28140625723842.

Multi-head attention (B=2, S=2048, D=1024, H=16, DH=64) with key-padding
mask, sharded over 8 NeuronCores as 2 batches x 4 head-groups (tensor
parallel over heads, data parallel over batch).

Per-core strategy:
  - Host passes transposed activations qT/kT/vT [D, *] (bf16) so the
    d-contraction sits on SBUF partitions; k/v token columns are gathered
    down to the unmasked set (padded to a multiple of 128) — masked keys
    contribute exactly 0 to softmax numerator and denominator, so the
    result is unchanged while scores/exp/PV work halves.
  - Projections produce QT/KT transposed [dh, tokens] (2 heads stacked per
    128 partitions) and V natural [tokens, 4*(dh+1)] with a ones column
    per head.
  - scoresT[k, q] = KT_tile.T @ QT; the pad-key mask becomes a
    per-partition bias fused into the ScalarE exp:
    expS = exp(scores/sqrt(D) + (-1e9)*(1-mask)).
  - P@V uses lhsT = [V_h | 1] so the softmax denominator (row sum) comes
    out as column dh of the matmul output; a PE transpose brings each
    [65, 128] block to [q, 65] layout, where DVE reciprocal +
    tensor_scalar_mul normalize straight into the fp32 output buffer.
  - All matmul operands are bf16 (PSUM accumulation fp32); output fp32.
"""

import numpy as np

B, S, D, H = 2, 2048, 1024, 16
DH = D // H            # 64 head dim
NCORES = 8
GROUPS = NCORES // B   # 4 head groups
HL = H // GROUPS       # 4 heads per core
GW = HL * DH           # 256 output columns per core

P = 128
ND = D // P            # 8 contraction tiles
NT = S // P            # 16 q token tiles
QB = 1024              # q block (one exp op width)
NQB = S // QB          # 2
CH = 512               # matmul free-dim chunk (one PSUM bank fp32)
NCH = QB // CH         # 2

COMPACT = True         # gather unmasked k/v tokens on host

_CACHE = {}


def _chunks(total, width):
    out = []
    o = 0
    while o < total:
        w = min(width, total - o)
        out.append((o, w))
        o += w
    return out


def _build_nc(nk, use_bias=True, debug=False, pv_inter=False):
    import concourse.bacc as bacc
    import concourse.mybir as mybir
    import concourse.tile as tile
    from concourse.masks import make_identity

    f32 = mybir.dt.float32
    bf16 = mybir.dt.bfloat16
    i32 = mybir.dt.int32
    Exp = mybir.ActivationFunctionType.Exp
    SCALE = float(1.0 / np.sqrt(np.float32(D)))
    NTK = nk // P          # k token tiles (compacted)

    nc = bacc.Bacc(None, target_bir_lowering=False)
    qt_d = nc.dram_tensor("qt", [D, S], bf16, kind="ExternalInput")
    kt_d = nc.dram_tensor("kt", [D, nk], bf16, kind="ExternalInput")
    vt_d = nc.dram_tensor("vt", [D, nk], bf16, kind="ExternalInput")
    wq_d = nc.dram_tensor("wq", [D, GW], bf16, kind="ExternalInput")
    wk_d = nc.dram_tensor("wk", [D, GW], bf16, kind="ExternalInput")
    wv_d = nc.dram_tensor("wv", [D, GW], bf16, kind="ExternalInput")
    bq_d = nc.dram_tensor("bq", [GW], bf16, kind="ExternalInput")
    bk_d = nc.dram_tensor("bk", [GW], bf16, kind="ExternalInput")
    bv_d = nc.dram_tensor("bv", [GW], bf16, kind="ExternalInput")
    mask_d = nc.dram_tensor("mask", [nk], i32, kind="ExternalInput")
    out_d = nc.dram_tensor("out", [S, GW], f32, kind="ExternalOutput")
    if debug:
        dbg_qt = nc.dram_tensor("dbg_qt", [P, HL // 2, S], bf16, kind="ExternalOutput")
        dbg_kt = nc.dram_tensor("dbg_kt", [P, HL // 2, nk], bf16, kind="ExternalOutput")
        dbg_v = nc.dram_tensor("dbg_v", [P, NTK, HL * (DH + 1)], bf16, kind="ExternalOutput")

    with tile.TileContext(nc) as tc:
        with (
            tc.tile_pool(name="consts", bufs=1) as consts,
            tc.tile_pool(name="persist", bufs=1) as persist,
            tc.tile_pool(name="wpool", bufs=2) as wpool,
            tc.tile_pool(name="xt", bufs=3) as xtp,
            tc.tile_pool(name="vx", bufs=ND) as vxp,
            tc.tile_pool(name="exps", bufs=2 * NTK + 2) as expp,
            tc.tile_pool(name="tmp", bufs=6) as tmpp,
            tc.tile_pool(name="rec", bufs=4) as recp,
        ):
            ident = consts.tile([P, P], f32, tag="ident")
            make_identity(nc, ident)
            ones = consts.tile([1, CH], bf16, tag="ones")
            nc.vector.memset(ones, 1.0)

            # mask[k] -> per-partition exp bias: (m - 1) * 1e9  (0 or -1e9)
            maski = consts.tile([P, NTK], i32, tag="maski")
            nc.scalar.dma_start(maski, mask_d.rearrange("(t p) -> p t", p=P))
            maskb = consts.tile([P, NTK], f32, tag="maskb")
            nc.vector.tensor_scalar(
                maskb, maski, -1.0, 1e9,
                mybir.AluOpType.add, mybir.AluOpType.mult,
            )

            brow = {}
            if use_bias:
                for nm, drm in (("q", bq_d), ("k", bk_d), ("v", bv_d)):
                    t = consts.tile([1, GW], bf16, tag=f"bias_{nm}")
                    nc.scalar.dma_start(t, drm[None, :])
                    brow[nm] = t

            QT = persist.tile([P, HL // 2, S], bf16, tag="QT")
            KT = persist.tile([P, HL // 2, nk], bf16, tag="KT")
            V = persist.tile([P, NTK, HL * (DH + 1)], bf16, tag="V")
            V4 = V.rearrange("p t (h e) -> p t h e", h=HL)
            out_sb = persist.tile([P, NT, GW], f32, tag="osb")

            for h in range(HL):
                nc.vector.memset(V4[:, :, h, DH], 1.0)

            with tc.tile_pool(name="pps", bufs=8, space="PSUM") as pps:
                # ---- QT / KT projections: out[dh2, tok] accumulated over d ----
                for nm, xdr, wdr, bkey, OUT, width in (
                    ("q", qt_d, wq_d, "q", QT, S),
                    ("k", kt_d, wk_d, "k", KT, nk),
                ):
                    w_sb = wpool.tile([P, ND, GW], bf16, tag="w")
                    wdr_blk = wdr.rearrange("(n p) w -> p n w", p=P)
                    chs = _chunks(width, CH)
                    pst = {}
                    for dt_ in range(ND):
                        nc.sync.dma_start(w_sb[:, dt_, :], wdr_blk[:, dt_, :])
                        x_sb = xtp.tile([P, S], bf16, tag="xt",
                                        name=f"x_{nm}_{dt_}")
                        if dt_ == 0:
                            half = (len(chs) + 1) // 2 * CH
                            half = min(half, width)
                            nc.sync.dma_start(x_sb[:, :half],
                                              xdr[dt_ * P:(dt_ + 1) * P, :half])
                            if half < width:
                                nc.sync.dma_start(
                                    x_sb[:, half:width],
                                    xdr[dt_ * P:(dt_ + 1) * P, half:])
                        else:
                            nc.sync.dma_start(x_sb[:, :width],
                                              xdr[dt_ * P:(dt_ + 1) * P, :])
                        for hp in range(HL // 2):
                            for ci, (co, cw) in enumerate(chs):
                                if dt_ == 0:
                                    pst[(hp, ci)] = pps.tile(
                                        [P, CH], f32, tag="pp",
                                        name=f"pp_{nm}_{hp}_{ci}")
                                nc.tensor.matmul(
                                    pst[(hp, ci)][:, :cw],
                                    lhsT=w_sb[:, dt_, hp * P:(hp + 1) * P],
                                    rhs=x_sb[:, co:co + cw],
                                    start=(dt_ == 0),
                                    stop=(not use_bias and dt_ == ND - 1),
                                )
                    for hp in range(HL // 2):
                        for ci, (co, cw) in enumerate(chs):
                            if use_bias:
                                nc.tensor.matmul(
                                    pst[(hp, ci)][:, :cw],
                                    lhsT=brow[bkey][:, hp * P:(hp + 1) * P],
                                    rhs=ones[:, :cw],
                                    start=False, stop=True,
                                )
                            nc.vector.tensor_copy(
                                out=OUT[:, hp, co:co + cw],
                                in_=pst[(hp, ci)][:, :cw],
                            )

                # ---- V projection: natural [tok, 4*dh] ----
                # tok-tile outer so each PSUM accumulation group owns a
                # whole bank (start=True clears has_written bank-wide).
                wv_sb = wpool.tile([P, ND, GW], bf16, tag="w")
                nc.sync.dma_start(wv_sb, wv_d.rearrange("(n p) w -> p n w", p=P))
                xvt = []
                for dt_ in range(ND):
                    t = vxp.tile([P, nk], bf16, tag="xvt", name=f"xvt_{dt_}")
                    nc.sync.dma_start(t, vt_d[dt_ * P:(dt_ + 1) * P, :])
                    xvt.append(t)
                for tt in range(NTK):
                    vp_ps = pps.tile([P, GW], f32, tag="pp", name=f"ppv_{tt}")
                    for dt_ in range(ND):
                        nc.tensor.matmul(
                            vp_ps,
                            lhsT=xvt[dt_][:, tt * P:(tt + 1) * P],
                            rhs=wv_sb[:, dt_, :],
                            start=(dt_ == 0),
                            stop=(not use_bias and dt_ == ND - 1),
                        )
                    if use_bias:
                        nc.tensor.matmul(
                            vp_ps,
                            lhsT=ones[:, :P],
                            rhs=brow["v"],
                            start=False, stop=True,
                        )
                    nc.vector.tensor_copy(
                        out=V4[:, tt, :, :DH],
                        in_=vp_ps.rearrange("p (h e) -> p h e", h=HL),
                    )

            # ---- attention ----
            with (
                tc.tile_pool(name="pss", bufs=2, space="PSUM") as pss,
                tc.tile_pool(name="pspv", bufs=2, space="PSUM") as pspv,
                tc.tile_pool(name="pstr", bufs=2, space="PSUM") as pstr,
            ):
                out_blk = out_d.rearrange("(t p) w -> p t w", p=P)

                def emit_normalize(items):
                    # deferred tail of a block: PE transposes + DVE normalize
                    for pv_sb, hh, qq, c in items:
                        for q4 in range(CH // P):
                            tps = pstr.tile([P, DH + 1], f32, tag="tr")
                            nc.tensor.transpose(
                                tps,
                                pv_sb[:, q4 * P:(q4 + 1) * P],
                                ident[:DH + 1, :DH + 1],
                            )
                            rec = recp.tile([P, 1], f32, tag="rec")
                            nc.vector.reciprocal(rec, tps[:, DH:DH + 1])
                            tokt = qq * (QB // P) + c * (CH // P) + q4
                            nc.vector.tensor_scalar_mul(
                                out_sb[:, tokt, hh * DH:(hh + 1) * DH],
                                tps[:, :DH],
                                rec,
                            )
                        if hh == HL - 1:
                            t0 = qq * (QB // P) + c * (CH // P)
                            t1 = t0 + CH // P
                            nc.sync.dma_start(
                                out_blk[:, t0:t1, :], out_sb[:, t0:t1, :]
                            )

                pending = []
                for h in range(HL):
                    hp, ho = divmod(h, 2)
                    po = ho * DH  # partition offset within the stacked pair
                    for qb in range(NQB):
                        etiles = []
                        for kt_ in range(NTK):
                            sps = pss.tile([P, QB], f32, tag="s",
                                           name=f"s_{h}_{qb}_{kt_}")
                            for c in range(NCH):
                                nc.tensor.matmul(
                                    sps[:, c * CH:(c + 1) * CH],
                                    lhsT=KT[po:po + DH, hp, kt_ * P:(kt_ + 1) * P],
                                    rhs=QT[po:po + DH, hp,
                                           qb * QB + c * CH:qb * QB + (c + 1) * CH],
                                    start=True, stop=True,
                                )
                            e = expp.tile([P, QB], bf16, tag="e",
                                          name=f"e_{h}_{qb}_{kt_}")
                            nc.scalar.activation(
                                e, sps, Exp,
                                bias=maskb[:, kt_:kt_ + 1], scale=SCALE,
                            )
                            etiles.append(e)
                            if kt_ == 1 and pending:
                                emit_normalize(pending)
                                pending = []
                            if kt_ == 0:
                                pvt = [pspv.tile([DH + 1, CH], f32, tag="pv",
                                                 name=f"pv_{h}_{qb}_{c}")
                                       for c in range(NCH)]
                            kp = kt_ - 2
                            if pv_inter and kp >= 0:
                                for c in range(NCH):
                                    nc.tensor.matmul(
                                        pvt[c],
                                        lhsT=V[:, kp, h * (DH + 1):(h + 1) * (DH + 1)],
                                        rhs=etiles[kp][:, c * CH:(c + 1) * CH],
                                        start=(kp == 0), stop=False,
                                    )
                        for kp in (range(NTK - 2, NTK) if pv_inter
                                   else range(NTK)):
                            for c in range(NCH):
                                nc.tensor.matmul(
                                    pvt[c],
                                    lhsT=V[:, kp, h * (DH + 1):(h + 1) * (DH + 1)],
                                    rhs=etiles[kp][:, c * CH:(c + 1) * CH],
                                    start=(kp == 0), stop=(kp == NTK - 1),
                                )
                        for c in range(NCH):
                            pv_sb = tmpp.tile([DH + 1, CH], f32, tag="pvsb",
                                              name=f"pvsb_{h}_{qb}_{c}")
                            nc.vector.tensor_copy(out=pv_sb, in_=pvt[c])
                            pending.append((pv_sb, h, qb, c))
                emit_normalize(pending)

                if debug:
                    nc.sync.dma_start(dbg_qt[:], QT)
                    nc.sync.dma_start(dbg_kt[:], KT)
                    nc.sync.dma_start(dbg_v[:], V)
    nc.compile()
    return nc


def _get_nc(nk, use_bias=True, debug=False, pv_inter=False):
    key = (nk, use_bias, debug, pv_inter)
    if key not in _CACHE:
        _CACHE[key] = _build_nc(nk, use_bias=use_bias, debug=debug,
                                pv_inter=pv_inter)
    return _CACHE[key]


def _run(nc, in_maps, trace=False):
    from concourse.bass_utils import run_bass_kernel_spmd

    return run_bass_kernel_spmd(
        nc, in_maps, core_ids=list(range(NCORES)), trace=trace
    )


def _make_in_maps(q, k, v, mask, Wq, bq, Wk, bk, Wv, bv):
    import ml_dtypes

    bf16 = ml_dtypes.bfloat16
    q = np.asarray(q, np.float32)
    k = np.asarray(k, np.float32)
    v = np.asarray(v, np.float32)
    mask = np.asarray(mask, np.int32)
    Wq = np.asarray(Wq, np.float32).astype(bf16)
    Wk = np.asarray(Wk, np.float32).astype(bf16)
    Wv = np.asarray(Wv, np.float32).astype(bf16)
    bq = np.asarray(bq, np.float32).astype(bf16)
    bk = np.asarray(bk, np.float32).astype(bf16)
    bv = np.asarray(bv, np.float32).astype(bf16)

    use_bias = bool(
        np.any(np.asarray(bq, np.float32))
        or np.any(np.asarray(bk, np.float32))
        or np.any(np.asarray(bv, np.float32))
    )
    if COMPACT:
        idxs = [np.nonzero(mask[b])[0] for b in range(B)]
        neff = max(1, max(len(ix) for ix in idxs))
        nk = -(-neff // P) * P  # round up to multiple of 128
    else:
        idxs = [np.arange(S) for _ in range(B)]
        nk = S

    qT = [np.ascontiguousarray(q[b].T).astype(bf16) for b in range(B)]
    kT, vT, mk = [], [], []
    for b in range(B):
        ix = idxs[b]
        kc = np.zeros((D, nk), bf16)
        vc = np.zeros((D, nk), bf16)
        kc[:, :len(ix)] = k[b].T[:, ix].astype(bf16)
        vc[:, :len(ix)] = v[b].T[:, ix].astype(bf16)
        kT.append(kc)
        vT.append(vc)
        m = np.zeros((nk,), np.int32)
        if COMPACT:
            m[:len(ix)] = 1
        else:
            m[:] = mask[b]
        mk.append(m)

    in_maps = []
    for c in range(NCORES):
        b, g = divmod(c, GROUPS)
        cols = slice(g * GW, (g + 1) * GW)
        in_maps.append({
            "qt": qT[b],
            "kt": kT[b],
            "vt": vT[b],
            "wq": np.ascontiguousarray(Wq[:, cols]),
            "wk": np.ascontiguousarray(Wk[:, cols]),
            "wv": np.ascontiguousarray(Wv[:, cols]),
            "bq": np.ascontiguousarray(bq[cols]),
            "bk": np.ascontiguousarray(bk[cols]),
            "bv": np.ascontiguousarray(bv[cols]),
            "mask": mk[b],
        })
    return nk, use_bias, in_maps


def _assemble(results):
    out = np.empty((B, S, D), np.float32)
    for c in range(NCORES):
        b, g = divmod(c, GROUPS)
        out[b, :, g * GW:(g + 1) * GW] = results[c]["out"]
    return out


def kernel(q, k, v, mask, Wq, bq, Wk, bk, Wv, bv):
    nk, use_bias, in_maps = _make_in_maps(q, k, v, mask, Wq, bq, Wk, bk, Wv, bv)
    res = _run(_get_nc(nk, use_bias), in_maps, trace=False)
    return _assemble(res.results)


def _install_ntff_hook():
    """The image's antenv stub lacks axon_hooks; synthesize it and register
    the ctypes NTFF hook that trn_agent_boot would have installed."""
    import sys
    import types

    import antenv

    if "antenv.axon_hooks" in sys.modules:
        return
    mod = types.ModuleType("antenv.axon_hooks")
    state = {"hook": None}
    mod.set_axon_ntff_profile_hook = lambda h: state.__setitem__("hook", h)
    mod.get_axon_ntff_profile_hook = lambda: state["hook"]
    sys.modules["antenv.axon_hooks"] = mod
    antenv.axon_hooks = mod
    try:
        from trn_agent_boot.trn_boot import _ntff_profile_via_ctypes

        mod.set_axon_ntff_profile_hook(
            _ntff_profile_via_ctypes("/opt/axon/libaxon_pjrt.so")
        )
    except Exception as e:
        print(f"ntff hook registration failed: {e}")


def kernel_traced(q, k, v, mask, Wq, bq, Wk, bk, Wv, bv):
    """Same as kernel() but also returns (output, exec_time_ns)."""
    _install_ntff_hook()
    nk, use_bias, in_maps = _make_in_maps(q, k, v, mask, Wq, bq, Wk, bk, Wv, bv)
    res = _run(_get_nc(nk, use_bias), in_maps, trace=True)
    return _assemble(res.results), res.exec_time_ns


# revision 32
# speedup vs baseline: 1.1011x; 1.0500x over previous
"""Bass/Trainium2 kernel for nn_Attention_28140625723842.

Multi-head attention (B=2, S=2048, D=1024, H=16, DH=64) with key-padding
mask, sharded over 8 NeuronCores as 2 batches x 4 head-groups (tensor
parallel over heads, data parallel over batch).

Per-core strategy:
  - Host passes transposed activations qT/kT/vT [D, *] (bf16) so the
    d-contraction sits on SBUF partitions; k/v token columns are gathered
    down to the unmasked set (padded to a multiple of 128) — masked keys
    contribute exactly 0 to softmax numerator and denominator, so the
    result is unchanged while scores/exp/PV work halves.
  - Projections produce QT/KT transposed [dh, tokens] (2 heads stacked per
    128 partitions) and V natural [tokens, 4*(dh+1)] with a ones column
    per head.
  - scoresT[k, q] = KT_tile.T @ QT; the pad-key mask becomes a
    per-partition bias fused into the ScalarE exp:
    expS = exp(scores/sqrt(D) + (-1e9)*(1-mask)).
  - P@V uses lhsT = [V_h | 1] so the softmax denominator (row sum) comes
    out as column dh of the matmul output; a PE transpose brings each
    [65, 128] block to [q, 65] layout, where DVE reciprocal +
    tensor_scalar_mul normalize straight into the fp32 output buffer.
  - All matmul operands are bf16 (PSUM accumulation fp32); output fp32.
"""

import numpy as np

B, S, D, H = 2, 2048, 1024, 16
DH = D // H            # 64 head dim
NCORES = 8
GROUPS = NCORES // B   # 4 head groups
HL = H // GROUPS       # 4 heads per core
GW = HL * DH           # 256 output columns per core

P = 128
ND = D // P            # 8 contraction tiles
NT = S // P            # 16 q token tiles
QB = 1024              # q block (one exp op width)
NQB = S // QB          # 2
CH = 512               # matmul free-dim chunk (one PSUM bank fp32)
NCH = QB // CH         # 2

COMPACT = True         # gather unmasked k/v tokens on host

_CACHE = {}


def _chunks(total, width):
    out = []
    o = 0
    while o < total:
        w = min(width, total - o)
        out.append((o, w))
        o += w
    return out


def _build_nc(nk, use_bias=True, debug=False, pv_inter=False):
    import concourse.bacc as bacc
    import concourse.mybir as mybir
    import concourse.tile as tile
    from concourse.masks import make_identity

    f32 = mybir.dt.float32
    bf16 = mybir.dt.bfloat16
    i32 = mybir.dt.int32
    Exp = mybir.ActivationFunctionType.Exp
    SCALE = float(1.0 / np.sqrt(np.float32(D)))
    NTK = nk // P          # k token tiles (compacted)

    nc = bacc.Bacc(None, target_bir_lowering=False)
    qt_d = nc.dram_tensor("qt", [D, S], bf16, kind="ExternalInput")
    kt_d = nc.dram_tensor("kt", [D, nk], bf16, kind="ExternalInput")
    vt_d = nc.dram_tensor("vt", [D, nk], bf16, kind="ExternalInput")
    wq_d = nc.dram_tensor("wq", [D, GW], bf16, kind="ExternalInput")
    wk_d = nc.dram_tensor("wk", [D, GW], bf16, kind="ExternalInput")
    wv_d = nc.dram_tensor("wv", [D, GW], bf16, kind="ExternalInput")
    bq_d = nc.dram_tensor("bq", [GW], bf16, kind="ExternalInput")
    bk_d = nc.dram_tensor("bk", [GW], bf16, kind="ExternalInput")
    bv_d = nc.dram_tensor("bv", [GW], bf16, kind="ExternalInput")
    mask_d = nc.dram_tensor("mask", [nk], i32, kind="ExternalInput")
    out_d = nc.dram_tensor("out", [S, GW], f32, kind="ExternalOutput")
    if debug:
        dbg_qt = nc.dram_tensor("dbg_qt", [P, HL // 2, S], bf16, kind="ExternalOutput")
        dbg_kt = nc.dram_tensor("dbg_kt", [P, HL // 2, nk], bf16, kind="ExternalOutput")
        dbg_v = nc.dram_tensor("dbg_v", [P, NTK, HL * (DH + 1)], bf16, kind="ExternalOutput")

    with tile.TileContext(nc) as tc:
        with (
            tc.tile_pool(name="consts", bufs=1) as consts,
            tc.tile_pool(name="persist", bufs=1) as persist,
            tc.tile_pool(name="wpool", bufs=2) as wpool,
            tc.tile_pool(name="xt", bufs=5) as xtp,
            tc.tile_pool(name="vx", bufs=ND) as vxp,
            tc.tile_pool(name="exps", bufs=2 * NTK + 2) as expp,
            tc.tile_pool(name="tmp", bufs=6) as tmpp,
            tc.tile_pool(name="rec", bufs=4) as recp,
        ):
            ident = consts.tile([P, P], f32, tag="ident")
            make_identity(nc, ident)
            ones = consts.tile([1, CH], bf16, tag="ones")
            nc.vector.memset(ones, 1.0)

            # mask[k] -> per-partition exp bias: (m - 1) * 1e9  (0 or -1e9)
            maski = consts.tile([P, NTK], i32, tag="maski")
            nc.scalar.dma_start(maski, mask_d.rearrange("(t p) -> p t", p=P))
            maskb = consts.tile([P, NTK], f32, tag="maskb")
            nc.vector.tensor_scalar(
                maskb, maski, -1.0, 1e9,
                mybir.AluOpType.add, mybir.AluOpType.mult,
            )

            brow = {}
            if use_bias:
                for nm, drm in (("q", bq_d), ("k", bk_d), ("v", bv_d)):
                    t = consts.tile([1, GW], bf16, tag=f"bias_{nm}")
                    nc.scalar.dma_start(t, drm[None, :])
                    brow[nm] = t

            QT = persist.tile([P, HL // 2, S], bf16, tag="QT")
            KT = persist.tile([P, HL // 2, nk], bf16, tag="KT")
            V = persist.tile([P, NTK, HL * (DH + 1)], bf16, tag="V")
            V4 = V.rearrange("p t (h e) -> p t h e", h=HL)
            out_sb = persist.tile([P, NT, GW], f32, tag="osb")

            for h in range(HL):
                nc.vector.memset(V4[:, :, h, DH], 1.0)

            # V-projection activations: prefetch on the scalar HWDGE ring so
            # they stream concurrently with the sync-ring Q/K loads.
            xvt = []
            for dt_ in range(ND):
                t = vxp.tile([P, nk], bf16, tag="xvt", name=f"xvt_{dt_}")
                nc.scalar.dma_start(t, vt_d[dt_ * P:(dt_ + 1) * P, :])
                xvt.append(t)

            with tc.tile_pool(name="pps", bufs=8, space="PSUM") as pps:
                # ---- QT / KT projections: out[dh2, tok] accumulated over d ----
                for nm, xdr, wdr, bkey, OUT, width in (
                    ("q", qt_d, wq_d, "q", QT, S),
                    ("k", kt_d, wk_d, "k", KT, nk),
                ):
                    w_sb = wpool.tile([P, ND, GW], bf16, tag="w")
                    wdr_blk = wdr.rearrange("(n p) w -> p n w", p=P)
                    chs = _chunks(width, CH)
                    pst = {}
                    for dt_ in range(ND):
                        nc.sync.dma_start(w_sb[:, dt_, :], wdr_blk[:, dt_, :])
                        x_sb = xtp.tile([P, S], bf16, tag="xt",
                                        name=f"x_{nm}_{dt_}")
                        if dt_ == 0:
                            half = (len(chs) + 1) // 2 * CH
                            half = min(half, width)
                            nc.sync.dma_start(x_sb[:, :half],
                                              xdr[dt_ * P:(dt_ + 1) * P, :half])
                            if half < width:
                                nc.sync.dma_start(
                                    x_sb[:, half:width],
                                    xdr[dt_ * P:(dt_ + 1) * P, half:])
                        else:
                            nc.sync.dma_start(x_sb[:, :width],
                                              xdr[dt_ * P:(dt_ + 1) * P, :])
                        for hp in range(HL // 2):
                            for ci, (co, cw) in enumerate(chs):
                                if dt_ == 0:
                                    pst[(hp, ci)] = pps.tile(
                                        [P, CH], f32, tag="pp",
                                        name=f"pp_{nm}_{hp}_{ci}")
                                nc.tensor.matmul(
                                    pst[(hp, ci)][:, :cw],
                                    lhsT=w_sb[:, dt_, hp * P:(hp + 1) * P],
                                    rhs=x_sb[:, co:co + cw],
                                    start=(dt_ == 0),
                                    stop=(not use_bias and dt_ == ND - 1),
                                )
                    for hp in range(HL // 2):
                        for ci, (co, cw) in enumerate(chs):
                            if use_bias:
                                nc.tensor.matmul(
                                    pst[(hp, ci)][:, :cw],
                                    lhsT=brow[bkey][:, hp * P:(hp + 1) * P],
                                    rhs=ones[:, :cw],
                                    start=False, stop=True,
                                )
                            nc.vector.tensor_copy(
                                out=OUT[:, hp, co:co + cw],
                                in_=pst[(hp, ci)][:, :cw],
                            )

                # ---- V projection: natural [tok, 4*dh] ----
                # tok-tile outer so each PSUM accumulation group owns a
                # whole bank (start=True clears has_written bank-wide).
                wv_sb = wpool.tile([P, ND, GW], bf16, tag="w")
                nc.sync.dma_start(wv_sb, wv_d.rearrange("(n p) w -> p n w", p=P))
                for tt in range(NTK):
                    vp_ps = pps.tile([P, GW], f32, tag="pp", name=f"ppv_{tt}")
                    for dt_ in range(ND):
                        nc.tensor.matmul(
                            vp_ps,
                            lhsT=xvt[dt_][:, tt * P:(tt + 1) * P],
                            rhs=wv_sb[:, dt_, :],
                            start=(dt_ == 0),
                            stop=(not use_bias and dt_ == ND - 1),
                        )
                    if use_bias:
                        nc.tensor.matmul(
                            vp_ps,
                            lhsT=ones[:, :P],
                            rhs=brow["v"],
                            start=False, stop=True,
                        )
                    nc.vector.tensor_copy(
                        out=V4[:, tt, :, :DH],
                        in_=vp_ps.rearrange("p (h e) -> p h e", h=HL),
                    )

            # ---- attention ----
            with (
                tc.tile_pool(name="pss", bufs=2, space="PSUM") as pss,
                tc.tile_pool(name="pspv", bufs=2, space="PSUM") as pspv,
                tc.tile_pool(name="pstr", bufs=2, space="PSUM") as pstr,
            ):
                out_blk = out_d.rearrange("(t p) w -> p t w", p=P)

                def emit_normalize(items):
                    # deferred tail of a block: PE transposes + DVE normalize
                    for pv_sb, hh, qq, c in items:
                        for q4 in range(CH // P):
                            tps = pstr.tile([P, DH + 1], f32, tag="tr")
                            nc.tensor.transpose(
                                tps,
                                pv_sb[:, q4 * P:(q4 + 1) * P],
                                ident[:DH + 1, :DH + 1],
                            )
                            rec = recp.tile([P, 1], f32, tag="rec")
                            nc.vector.reciprocal(rec, tps[:, DH:DH + 1])
                            tokt = qq * (QB // P) + c * (CH // P) + q4
                            nc.vector.tensor_scalar_mul(
                                out_sb[:, tokt, hh * DH:(hh + 1) * DH],
                                tps[:, :DH],
                                rec,
                            )
                        if hh == HL - 1:
                            t0 = qq * (QB // P) + c * (CH // P)
                            t1 = t0 + CH // P
                            nc.sync.dma_start(
                                out_blk[:, t0:t1, :], out_sb[:, t0:t1, :]
                            )

                pending = []
                for h in range(HL):
                    hp, ho = divmod(h, 2)
                    po = ho * DH  # partition offset within the stacked pair
                    for qb in range(NQB):
                        etiles = []
                        for kt_ in range(NTK):
                            sps = pss.tile([P, QB], f32, tag="s",
                                           name=f"s_{h}_{qb}_{kt_}")
                            for c in range(NCH):
                                nc.tensor.matmul(
                                    sps[:, c * CH:(c + 1) * CH],
                                    lhsT=KT[po:po + DH, hp, kt_ * P:(kt_ + 1) * P],
                                    rhs=QT[po:po + DH, hp,
                                           qb * QB + c * CH:qb * QB + (c + 1) * CH],
                                    start=True, stop=True,
                                )
                            e = expp.tile([P, QB], bf16, tag="e",
                                          name=f"e_{h}_{qb}_{kt_}")
                            nc.scalar.activation(
                                e, sps, Exp,
                                bias=maskb[:, kt_:kt_ + 1], scale=SCALE,
                            )
                            etiles.append(e)
                            if kt_ == 1 and pending:
                                emit_normalize(pending)
                                pending = []
                            if kt_ == 0:
                                pvt = [pspv.tile([DH + 1, CH], f32, tag="pv",
                                                 name=f"pv_{h}_{qb}_{c}")
                                       for c in range(NCH)]
                            kp = kt_ - 2
                            if pv_inter and kp >= 0:
                                for c in range(NCH):
                                    nc.tensor.matmul(
                                        pvt[c],
                                        lhsT=V[:, kp, h * (DH + 1):(h + 1) * (DH + 1)],
                                        rhs=etiles[kp][:, c * CH:(c + 1) * CH],
                                        start=(kp == 0), stop=False,
                                    )
                        for kp in (range(NTK - 2, NTK) if pv_inter
                                   else range(NTK)):
                            for c in range(NCH):
                                nc.tensor.matmul(
                                    pvt[c],
                                    lhsT=V[:, kp, h * (DH + 1):(h + 1) * (DH + 1)],
                                    rhs=etiles[kp][:, c * CH:(c + 1) * CH],
                                    start=(kp == 0), stop=(kp == NTK - 1),
                                )
                        for c in range(NCH):
                            pv_sb = tmpp.tile([DH + 1, CH], f32, tag="pvsb",
                                              name=f"pvsb_{h}_{qb}_{c}")
                            nc.vector.tensor_copy(out=pv_sb, in_=pvt[c])
                            pending.append((pv_sb, h, qb, c))
                emit_normalize(pending)

                if debug:
                    nc.sync.dma_start(dbg_qt[:], QT)
                    nc.sync.dma_start(dbg_kt[:], KT)
                    nc.sync.dma_start(dbg_v[:], V)
    nc.compile()
    return nc


def _get_nc(nk, use_bias=True, debug=False, pv_inter=False):
    key = (nk, use_bias, debug, pv_inter)
    if key not in _CACHE:
        _CACHE[key] = _build_nc(nk, use_bias=use_bias, debug=debug,
                                pv_inter=pv_inter)
    return _CACHE[key]


def _run(nc, in_maps, trace=False):
    from concourse.bass_utils import run_bass_kernel_spmd

    return run_bass_kernel_spmd(
        nc, in_maps, core_ids=list(range(NCORES)), trace=trace
    )


def _make_in_maps(q, k, v, mask, Wq, bq, Wk, bk, Wv, bv):
    import ml_dtypes

    bf16 = ml_dtypes.bfloat16
    q = np.asarray(q, np.float32)
    k = np.asarray(k, np.float32)
    v = np.asarray(v, np.float32)
    mask = np.asarray(mask, np.int32)
    Wq = np.asarray(Wq, np.float32).astype(bf16)
    Wk = np.asarray(Wk, np.float32).astype(bf16)
    Wv = np.asarray(Wv, np.float32).astype(bf16)
    bq = np.asarray(bq, np.float32).astype(bf16)
    bk = np.asarray(bk, np.float32).astype(bf16)
    bv = np.asarray(bv, np.float32).astype(bf16)

    use_bias = bool(
        np.any(np.asarray(bq, np.float32))
        or np.any(np.asarray(bk, np.float32))
        or np.any(np.asarray(bv, np.float32))
    )
    if COMPACT:
        idxs = [np.nonzero(mask[b])[0] for b in range(B)]
        neff = max(1, max(len(ix) for ix in idxs))
        nk = -(-neff // P) * P  # round up to multiple of 128
    else:
        idxs = [np.arange(S) for _ in range(B)]
        nk = S

    qT = [np.ascontiguousarray(q[b].T).astype(bf16) for b in range(B)]
    kT, vT, mk = [], [], []
    for b in range(B):
        ix = idxs[b]
        kc = np.zeros((D, nk), bf16)
        vc = np.zeros((D, nk), bf16)
        kc[:, :len(ix)] = k[b].T[:, ix].astype(bf16)
        vc[:, :len(ix)] = v[b].T[:, ix].astype(bf16)
        kT.append(kc)
        vT.append(vc)
        m = np.zeros((nk,), np.int32)
        if COMPACT:
            m[:len(ix)] = 1
        else:
            m[:] = mask[b]
        mk.append(m)

    in_maps = []
    for c in range(NCORES):
        b, g = divmod(c, GROUPS)
        cols = slice(g * GW, (g + 1) * GW)
        in_maps.append({
            "qt": qT[b],
            "kt": kT[b],
            "vt": vT[b],
            "wq": np.ascontiguousarray(Wq[:, cols]),
            "wk": np.ascontiguousarray(Wk[:, cols]),
            "wv": np.ascontiguousarray(Wv[:, cols]),
            "bq": np.ascontiguousarray(bq[cols]),
            "bk": np.ascontiguousarray(bk[cols]),
            "bv": np.ascontiguousarray(bv[cols]),
            "mask": mk[b],
        })
    return nk, use_bias, in_maps


def _assemble(results):
    out = np.empty((B, S, D), np.float32)
    for c in range(NCORES):
        b, g = divmod(c, GROUPS)
        out[b, :, g * GW:(g + 1) * GW] = results[c]["out"]
    return out


def kernel(q, k, v, mask, Wq, bq, Wk, bk, Wv, bv):
    nk, use_bias, in_maps = _make_in_maps(q, k, v, mask, Wq, bq, Wk, bk, Wv, bv)
    res = _run(_get_nc(nk, use_bias), in_maps, trace=False)
    return _assemble(res.results)


def _install_ntff_hook():
    """The image's antenv stub lacks axon_hooks; synthesize it and register
    the ctypes NTFF hook that trn_agent_boot would have installed."""
    import sys
    import types

    import antenv

    if "antenv.axon_hooks" in sys.modules:
        return
    mod = types.ModuleType("antenv.axon_hooks")
    state = {"hook": None}
    mod.set_axon_ntff_profile_hook = lambda h: state.__setitem__("hook", h)
    mod.get_axon_ntff_profile_hook = lambda: state["hook"]
    sys.modules["antenv.axon_hooks"] = mod
    antenv.axon_hooks = mod
    try:
        from trn_agent_boot.trn_boot import _ntff_profile_via_ctypes

        mod.set_axon_ntff_profile_hook(
            _ntff_profile_via_ctypes("/opt/axon/libaxon_pjrt.so")
        )
    except Exception as e:
        print(f"ntff hook registration failed: {e}")


def kernel_traced(q, k, v, mask, Wq, bq, Wk, bk, Wv, bv):
    """Same as kernel() but also returns (output, exec_time_ns)."""
    _install_ntff_hook()
    nk, use_bias, in_maps = _make_in_maps(q, k, v, mask, Wq, bq, Wk, bk, Wv, bv)
    res = _run(_get_nc(nk, use_bias), in_maps, trace=True)
    return _assemble(res.results), res.exec_time_ns


# revision 35
# speedup vs baseline: 1.1480x; 1.0426x over previous
"""Bass/Trainium2 kernel for nn_Attention_28140625723842.

Multi-head attention (B=2, S=2048, D=1024, H=16, DH=64) with key-padding
mask, sharded over 8 NeuronCores as 2 batches x 4 head-groups (tensor
parallel over heads, data parallel over batch).

Per-core strategy:
  - Host passes transposed activations qT/kT/vT [D, *] (bf16) so the
    d-contraction sits on SBUF partitions; k/v token columns are gathered
    down to the unmasked set (padded to a multiple of 128) — masked keys
    contribute exactly 0 to softmax numerator and denominator, so the
    result is unchanged while scores/exp/PV work halves.
  - Projections produce QT/KT transposed [dh, tokens] (2 heads stacked per
    128 partitions) and V natural [tokens, 4*(dh+1)] with a ones column
    per head.
  - scoresT[k, q] = KT_tile.T @ QT; the pad-key mask becomes a
    per-partition bias fused into the ScalarE exp:
    expS = exp(scores/sqrt(D) + (-1e9)*(1-mask)).
  - P@V uses lhsT = [V_h | 1] so the softmax denominator (row sum) comes
    out as column dh of the matmul output; a PE transpose brings each
    [65, 128] block to [q, 65] layout, where DVE reciprocal +
    tensor_scalar_mul normalize straight into the fp32 output buffer.
  - All matmul operands are bf16 (PSUM accumulation fp32); output fp32.
"""

import numpy as np

B, S, D, H = 2, 2048, 1024, 16
DH = D // H            # 64 head dim
NCORES = 8
GROUPS = NCORES // B   # 4 head groups
HL = H // GROUPS       # 4 heads per core
GW = HL * DH           # 256 output columns per core

P = 128
ND = D // P            # 8 contraction tiles
NT = S // P            # 16 q token tiles
QB = 1024              # q block (one exp op width)
NQB = S // QB          # 2
CH = 512               # matmul free-dim chunk (one PSUM bank fp32)
NCH = QB // CH         # 2

COMPACT = True         # gather unmasked k/v tokens on host

_CACHE = {}


def _chunks(total, width):
    out = []
    o = 0
    while o < total:
        w = min(width, total - o)
        out.append((o, w))
        o += w
    return out


def _build_nc(nk, use_bias=True, debug=False, pv_inter=False, w_scalar=True):
    import concourse.bacc as bacc
    import concourse.mybir as mybir
    import concourse.tile as tile
    from concourse.masks import make_identity

    f32 = mybir.dt.float32
    bf16 = mybir.dt.bfloat16
    i32 = mybir.dt.int32
    Exp = mybir.ActivationFunctionType.Exp
    SCALE = float(1.0 / np.sqrt(np.float32(D)))
    NTK = nk // P          # k token tiles (compacted)

    nc = bacc.Bacc(None, target_bir_lowering=False)
    qt_d = nc.dram_tensor("qt", [D, S], bf16, kind="ExternalInput")
    kt_d = nc.dram_tensor("kt", [D, nk], bf16, kind="ExternalInput")
    vt_d = nc.dram_tensor("vt", [D, nk], bf16, kind="ExternalInput")
    wq_d = nc.dram_tensor("wq", [D, GW], bf16, kind="ExternalInput")
    wk_d = nc.dram_tensor("wk", [D, GW], bf16, kind="ExternalInput")
    wv_d = nc.dram_tensor("wv", [D, GW], bf16, kind="ExternalInput")
    bq_d = nc.dram_tensor("bq", [GW], bf16, kind="ExternalInput")
    bk_d = nc.dram_tensor("bk", [GW], bf16, kind="ExternalInput")
    bv_d = nc.dram_tensor("bv", [GW], bf16, kind="ExternalInput")
    mask_d = nc.dram_tensor("mask", [nk], i32, kind="ExternalInput")
    out_d = nc.dram_tensor("out", [S, GW], f32, kind="ExternalOutput")
    if debug:
        dbg_qt = nc.dram_tensor("dbg_qt", [P, HL // 2, S], bf16, kind="ExternalOutput")
        dbg_kt = nc.dram_tensor("dbg_kt", [P, HL // 2, nk], bf16, kind="ExternalOutput")
        dbg_v = nc.dram_tensor("dbg_v", [P, NTK, HL * (DH + 1)], bf16, kind="ExternalOutput")

    with tile.TileContext(nc) as tc:
        with (
            tc.tile_pool(name="consts", bufs=1) as consts,
            tc.tile_pool(name="persist", bufs=1) as persist,
            tc.tile_pool(name="wpool", bufs=2) as wpool,
            tc.tile_pool(name="xt", bufs=5) as xtp,
            tc.tile_pool(name="vx", bufs=ND) as vxp,
            tc.tile_pool(name="exps", bufs=2 * NTK + 2) as expp,
            tc.tile_pool(name="tmp", bufs=6) as tmpp,
            tc.tile_pool(name="rec", bufs=4) as recp,
        ):
            ident = consts.tile([P, P], f32, tag="ident")
            make_identity(nc, ident)
            ones = consts.tile([1, CH], bf16, tag="ones")
            nc.vector.memset(ones, 1.0)

            # mask[k] -> per-partition exp bias: (m - 1) * 1e9  (0 or -1e9)
            maski = consts.tile([P, NTK], i32, tag="maski")
            nc.scalar.dma_start(maski, mask_d.rearrange("(t p) -> p t", p=P))
            maskb = consts.tile([P, NTK], f32, tag="maskb")
            nc.vector.tensor_scalar(
                maskb, maski, -1.0, 1e9,
                mybir.AluOpType.add, mybir.AluOpType.mult,
            )

            brow = {}
            if use_bias:
                for nm, drm in (("q", bq_d), ("k", bk_d), ("v", bv_d)):
                    t = consts.tile([1, GW], bf16, tag=f"bias_{nm}")
                    nc.scalar.dma_start(t, drm[None, :])
                    brow[nm] = t

            QT = persist.tile([P, HL // 2, S], bf16, tag="QT")
            KT = persist.tile([P, HL // 2, nk], bf16, tag="KT")
            V = persist.tile([P, NTK, HL * (DH + 1)], bf16, tag="V")
            V4 = V.rearrange("p t (h e) -> p t h e", h=HL)
            out_sb = persist.tile([P, NT, GW], f32, tag="osb")

            for h in range(HL):
                nc.vector.memset(V4[:, :, h, DH], 1.0)

            xvt = []

            with tc.tile_pool(name="pps", bufs=8, space="PSUM") as pps:
                # ---- QT / KT projections: out[dh2, tok] accumulated over d ----
                for nm, xdr, wdr, bkey, OUT, width in (
                    ("q", qt_d, wq_d, "q", QT, S),
                    ("k", kt_d, wk_d, "k", KT, nk),
                ):
                    if nm == "k":
                        # V-projection activations on the scalar HWDGE ring:
                        # streams during late-Q/K compute without competing
                        # with the Q loads for HBM bandwidth.
                        for dt_ in range(ND):
                            t = vxp.tile([P, nk], bf16, tag="xvt",
                                         name=f"xvt_{dt_}")
                            nc.scalar.dma_start(t, vt_d[dt_ * P:(dt_ + 1) * P, :])
                            xvt.append(t)
                    w_sb = wpool.tile([P, ND, GW], bf16, tag="w")
                    wdr_blk = wdr.rearrange("(n p) w -> p n w", p=P)
                    chs = _chunks(width, CH)
                    pst = {}
                    weng = nc.scalar if w_scalar else nc.sync
                    for dt_ in range(ND):
                        weng.dma_start(w_sb[:, dt_, :], wdr_blk[:, dt_, :])
                        x_sb = xtp.tile([P, S], bf16, tag="xt",
                                        name=f"x_{nm}_{dt_}")
                        if dt_ == 0:
                            half = (len(chs) + 1) // 2 * CH
                            half = min(half, width)
                            nc.sync.dma_start(x_sb[:, :half],
                                              xdr[dt_ * P:(dt_ + 1) * P, :half])
                            if half < width:
                                nc.sync.dma_start(
                                    x_sb[:, half:width],
                                    xdr[dt_ * P:(dt_ + 1) * P, half:])
                        else:
                            nc.sync.dma_start(x_sb[:, :width],
                                              xdr[dt_ * P:(dt_ + 1) * P, :])
                        for hp in range(HL // 2):
                            for ci, (co, cw) in enumerate(chs):
                                if dt_ == 0:
                                    pst[(hp, ci)] = pps.tile(
                                        [P, CH], f32, tag="pp",
                                        name=f"pp_{nm}_{hp}_{ci}")
                                nc.tensor.matmul(
                                    pst[(hp, ci)][:, :cw],
                                    lhsT=w_sb[:, dt_, hp * P:(hp + 1) * P],
                                    rhs=x_sb[:, co:co + cw],
                                    start=(dt_ == 0),
                                    stop=(not use_bias and dt_ == ND - 1),
                                )
                    for hp in range(HL // 2):
                        for ci, (co, cw) in enumerate(chs):
                            if use_bias:
                                nc.tensor.matmul(
                                    pst[(hp, ci)][:, :cw],
                                    lhsT=brow[bkey][:, hp * P:(hp + 1) * P],
                                    rhs=ones[:, :cw],
                                    start=False, stop=True,
                                )
                            nc.vector.tensor_copy(
                                out=OUT[:, hp, co:co + cw],
                                in_=pst[(hp, ci)][:, :cw],
                            )

                # ---- V projection: natural [tok, 4*dh] ----
                # tok-tile outer so each PSUM accumulation group owns a
                # whole bank (start=True clears has_written bank-wide).
                wv_sb = wpool.tile([P, ND, GW], bf16, tag="w")
                (nc.scalar if w_scalar else nc.sync).dma_start(wv_sb, wv_d.rearrange("(n p) w -> p n w", p=P))
                for tt in range(NTK):
                    vp_ps = pps.tile([P, GW], f32, tag="pp", name=f"ppv_{tt}")
                    for dt_ in range(ND):
                        nc.tensor.matmul(
                            vp_ps,
                            lhsT=xvt[dt_][:, tt * P:(tt + 1) * P],
                            rhs=wv_sb[:, dt_, :],
                            start=(dt_ == 0),
                            stop=(not use_bias and dt_ == ND - 1),
                        )
                    if use_bias:
                        nc.tensor.matmul(
                            vp_ps,
                            lhsT=ones[:, :P],
                            rhs=brow["v"],
                            start=False, stop=True,
                        )
                    nc.vector.tensor_copy(
                        out=V4[:, tt, :, :DH],
                        in_=vp_ps.rearrange("p (h e) -> p h e", h=HL),
                    )

            # ---- attention ----
            with (
                tc.tile_pool(name="pss", bufs=2, space="PSUM") as pss,
                tc.tile_pool(name="pspv", bufs=2, space="PSUM") as pspv,
                tc.tile_pool(name="pstr", bufs=2, space="PSUM") as pstr,
            ):
                out_blk = out_d.rearrange("(t p) w -> p t w", p=P)

                def emit_normalize(items):
                    # deferred tail of a block: PE transposes + DVE normalize
                    for pv_sb, hh, qq, c in items:
                        for q4 in range(CH // P):
                            tps = pstr.tile([P, DH + 1], f32, tag="tr")
                            nc.tensor.transpose(
                                tps,
                                pv_sb[:, q4 * P:(q4 + 1) * P],
                                ident[:DH + 1, :DH + 1],
                            )
                            rec = recp.tile([P, 1], f32, tag="rec")
                            nc.vector.reciprocal(rec, tps[:, DH:DH + 1])
                            tokt = qq * (QB // P) + c * (CH // P) + q4
                            nc.vector.tensor_scalar_mul(
                                out_sb[:, tokt, hh * DH:(hh + 1) * DH],
                                tps[:, :DH],
                                rec,
                            )
                        if hh == HL - 1:
                            t0 = qq * (QB // P) + c * (CH // P)
                            t1 = t0 + CH // P
                            nc.sync.dma_start(
                                out_blk[:, t0:t1, :], out_sb[:, t0:t1, :]
                            )

                pending = []
                for h in range(HL):
                    hp, ho = divmod(h, 2)
                    po = ho * DH  # partition offset within the stacked pair
                    for qb in range(NQB):
                        etiles = []
                        for kt_ in range(NTK):
                            sps = pss.tile([P, QB], f32, tag="s",
                                           name=f"s_{h}_{qb}_{kt_}")
                            for c in range(NCH):
                                nc.tensor.matmul(
                                    sps[:, c * CH:(c + 1) * CH],
                                    lhsT=KT[po:po + DH, hp, kt_ * P:(kt_ + 1) * P],
                                    rhs=QT[po:po + DH, hp,
                                           qb * QB + c * CH:qb * QB + (c + 1) * CH],
                                    start=True, stop=True,
                                )
                            e = expp.tile([P, QB], bf16, tag="e",
                                          name=f"e_{h}_{qb}_{kt_}")
                            nc.scalar.activation(
                                e, sps, Exp,
                                bias=maskb[:, kt_:kt_ + 1], scale=SCALE,
                            )
                            etiles.append(e)
                            if kt_ == 1 and pending:
                                emit_normalize(pending)
                                pending = []
                            if kt_ == 0:
                                pvt = [pspv.tile([DH + 1, CH], f32, tag="pv",
                                                 name=f"pv_{h}_{qb}_{c}")
                                       for c in range(NCH)]
                            kp = kt_ - 2
                            if pv_inter and kp >= 0:
                                for c in range(NCH):
                                    nc.tensor.matmul(
                                        pvt[c],
                                        lhsT=V[:, kp, h * (DH + 1):(h + 1) * (DH + 1)],
                                        rhs=etiles[kp][:, c * CH:(c + 1) * CH],
                                        start=(kp == 0), stop=False,
                                    )
                        for kp in (range(NTK - 2, NTK) if pv_inter
                                   else range(NTK)):
                            for c in range(NCH):
                                nc.tensor.matmul(
                                    pvt[c],
                                    lhsT=V[:, kp, h * (DH + 1):(h + 1) * (DH + 1)],
                                    rhs=etiles[kp][:, c * CH:(c + 1) * CH],
                                    start=(kp == 0), stop=(kp == NTK - 1),
                                )
                        for c in range(NCH):
                            pv_sb = tmpp.tile([DH + 1, CH], f32, tag="pvsb",
                                              name=f"pvsb_{h}_{qb}_{c}")
                            nc.vector.tensor_copy(out=pv_sb, in_=pvt[c])
                            pending.append((pv_sb, h, qb, c))
                emit_normalize(pending)

                if debug:
                    nc.sync.dma_start(dbg_qt[:], QT)
                    nc.sync.dma_start(dbg_kt[:], KT)
                    nc.sync.dma_start(dbg_v[:], V)
    nc.compile()
    return nc


def _get_nc(nk, use_bias=True, debug=False, pv_inter=False, w_scalar=True):
    key = (nk, use_bias, debug, pv_inter, w_scalar)
    if key not in _CACHE:
        _CACHE[key] = _build_nc(nk, use_bias=use_bias, debug=debug,
                                pv_inter=pv_inter, w_scalar=w_scalar)
    return _CACHE[key]


def _run(nc, in_maps, trace=False):
    from concourse.bass_utils import run_bass_kernel_spmd

    return run_bass_kernel_spmd(
        nc, in_maps, core_ids=list(range(NCORES)), trace=trace
    )


def _make_in_maps(q, k, v, mask, Wq, bq, Wk, bk, Wv, bv):
    import ml_dtypes

    bf16 = ml_dtypes.bfloat16
    q = np.asarray(q, np.float32)
    k = np.asarray(k, np.float32)
    v = np.asarray(v, np.float32)
    mask = np.asarray(mask, np.int32)
    Wq = np.asarray(Wq, np.float32).astype(bf16)
    Wk = np.asarray(Wk, np.float32).astype(bf16)
    Wv = np.asarray(Wv, np.float32).astype(bf16)
    bq = np.asarray(bq, np.float32).astype(bf16)
    bk = np.asarray(bk, np.float32).astype(bf16)
    bv = np.asarray(bv, np.float32).astype(bf16)

    use_bias = bool(
        np.any(np.asarray(bq, np.float32))
        or np.any(np.asarray(bk, np.float32))
        or np.any(np.asarray(bv, np.float32))
    )
    if COMPACT:
        idxs = [np.nonzero(mask[b])[0] for b in range(B)]
        neff = max(1, max(len(ix) for ix in idxs))
        nk = -(-neff // P) * P  # round up to multiple of 128
    else:
        idxs = [np.arange(S) for _ in range(B)]
        nk = S

    qT = [np.ascontiguousarray(q[b].T).astype(bf16) for b in range(B)]
    kT, vT, mk = [], [], []
    for b in range(B):
        ix = idxs[b]
        kc = np.zeros((D, nk), bf16)
        vc = np.zeros((D, nk), bf16)
        kc[:, :len(ix)] = k[b].T[:, ix].astype(bf16)
        vc[:, :len(ix)] = v[b].T[:, ix].astype(bf16)
        kT.append(kc)
        vT.append(vc)
        m = np.zeros((nk,), np.int32)
        if COMPACT:
            m[:len(ix)] = 1
        else:
            m[:] = mask[b]
        mk.append(m)

    in_maps = []
    for c in range(NCORES):
        b, g = divmod(c, GROUPS)
        cols = slice(g * GW, (g + 1) * GW)
        in_maps.append({
            "qt": qT[b],
            "kt": kT[b],
            "vt": vT[b],
            "wq": np.ascontiguousarray(Wq[:, cols]),
            "wk": np.ascontiguousarray(Wk[:, cols]),
            "wv": np.ascontiguousarray(Wv[:, cols]),
            "bq": np.ascontiguousarray(bq[cols]),
            "bk": np.ascontiguousarray(bk[cols]),
            "bv": np.ascontiguousarray(bv[cols]),
            "mask": mk[b],
        })
    return nk, use_bias, in_maps


def _assemble(results):
    out = np.empty((B, S, D), np.float32)
    for c in range(NCORES):
        b, g = divmod(c, GROUPS)
        out[b, :, g * GW:(g + 1) * GW] = results[c]["out"]
    return out


def kernel(q, k, v, mask, Wq, bq, Wk, bk, Wv, bv):
    nk, use_bias, in_maps = _make_in_maps(q, k, v, mask, Wq, bq, Wk, bk, Wv, bv)
    res = _run(_get_nc(nk, use_bias), in_maps, trace=False)
    return _assemble(res.results)


def _install_ntff_hook():
    """The image's antenv stub lacks axon_hooks; synthesize it and register
    the ctypes NTFF hook that trn_agent_boot would have installed."""
    import sys
    import types

    import antenv

    if "antenv.axon_hooks" in sys.modules:
        return
    mod = types.ModuleType("antenv.axon_hooks")
    state = {"hook": None}
    mod.set_axon_ntff_profile_hook = lambda h: state.__setitem__("hook", h)
    mod.get_axon_ntff_profile_hook = lambda: state["hook"]
    sys.modules["antenv.axon_hooks"] = mod
    antenv.axon_hooks = mod
    try:
        from trn_agent_boot.trn_boot import _ntff_profile_via_ctypes

        mod.set_axon_ntff_profile_hook(
            _ntff_profile_via_ctypes("/opt/axon/libaxon_pjrt.so")
        )
    except Exception as e:
        print(f"ntff hook registration failed: {e}")


def kernel_traced(q, k, v, mask, Wq, bq, Wk, bk, Wv, bv):
    """Same as kernel() but also returns (output, exec_time_ns)."""
    _install_ntff_hook()
    nk, use_bias, in_maps = _make_in_maps(q, k, v, mask, Wq, bq, Wk, bk, Wv, bv)
    res = _run(_get_nc(nk, use_bias), in_maps, trace=True)
    return _assemble(res.results), res.exec_time_ns
